# revision 1
# baseline (speedup 1.0000x reference)
"""Dilated-attention Trainium2 kernel (8 NeuronCores, SPMD).

Problem: x [4, 16384, 768] f32. Per 512-token segment, take every 2nd
position (dilation 2) -> 128 independent segments of [256, 768]; per-segment
self-attention out = softmax(xs @ xs.T / sqrt(768)) @ xs; output [4, 8192, 768].

Sharding: 128 (batch x segment) attention problems are fully independent ->
16 segments per core, no cross-core communication. Dilation gather is done
host-side while building each core's input slice (it is pure data movement).

Per-core device kernel, processed in pairs of segments (bigger DMAs). Per
segment (L=256 positions, D=768 features):
  1. DMA in  X as [128, t, 768] f32r tiles (positions t*128+p, features d)
     + a ones column appended at free offset D (for softmax denominators)
  2. cast X -> bf16 (VectorE), 12 PE transposes (bf16, fast-weight-load)
     -> XT[d] [128, 256] bf16 feature-major
  3. S^T tiles [128, 256] f32 = XT[d][:,kblk].T @ XT[d] accumulated over d
     (bf16: scores only steer the softmax; S is symmetric so these [k, q]
     tiles need no further transpose)
  4. exp on ScalarE with scale 1/sqrt(768): E[kt] = exp(S^T[kt] * scale),
     written as f32r
  5. out tiles = E[kt][:, qblk].T @ [X[kt] | ones] f32r accumulated over kt;
     the ones column makes out[:, 384] the softmax denominator for free
  6. recip[qt] = 1/den (VectorE), evict PSUM->SBUF scaled by recip[qt]
     (split across ScalarE/VectorE)
  7. DMA out

The V-side matmul stays f32r (tf32) so the output inherits ~1e-4 relative
error; the Q/K side in bf16 only perturbs attention logits.
"""

import numpy as np

import concourse.bass as bass
import concourse.mybir as mybir
import concourse.tile as tile
from concourse.bass_utils import run_bass_kernel_spmd
from concourse.masks import make_identity

F32 = mybir.dt.float32
R32 = mybir.dt.float32r  # tf32: full-rate PE matmul on f32 data
BF16 = mybir.dt.bfloat16

B, S_FULL, D = 4, 16384, 768
SEG, DIL = 512, 2
L = SEG // DIL                      # 256 positions per dilated segment
NSEG = B * (S_FULL // SEG)          # 128 segments total
NCORE = 8
SEG_PER_CORE = NSEG // NCORE        # 16
PAIR = 4                            # segments per DMA batch
NP = SEG_PER_CORE // PAIR           # 8 pairs
KT = L // 128                       # 2 position tiles per segment
TT = PAIR * KT                      # 4 position tiles per pair
DT = D // 128                       # 6 feature tiles
DW = D + 4                          # xn free pitch (col D holds ones)
SCALE = 1.0 / float(np.sqrt(D))


def build_nc():
    nc = bass.Bass()
    x = nc.dram_tensor("x", [SEG_PER_CORE, L, D], F32, kind="ExternalInput")
    y = nc.dram_tensor("y", [SEG_PER_CORE, L, D], F32, kind="ExternalOutput")

    with tile.TileContext(nc) as tc:
        with (
            tc.tile_pool(name="const", bufs=1) as const_pool,
            tc.tile_pool(name="xn", bufs=2) as xn_pool,
            tc.tile_pool(name="xb", bufs=2) as xb_pool,
            tc.tile_pool(name="xt", bufs=2 * PAIR * DT) as xt_pool,
            tc.tile_pool(name="e", bufs=2 * PAIR * KT) as e_pool,
            tc.tile_pool(name="recip", bufs=2 * PAIR * KT) as recip_pool,
            tc.tile_pool(name="osb", bufs=2) as osb_pool,
            tc.tile_pool(name="tp", bufs=3, space="PSUM") as tp_pool,
            tc.tile_pool(name="sps", bufs=2, space="PSUM") as s_pool,
            tc.tile_pool(name="op", bufs=3, space="PSUM") as out_pool,
        ):
            identity_f = const_pool.tile([128, 128], F32)
            make_identity(nc, identity_f[:])
            identity = const_pool.tile([128, 128], BF16)
            nc.vector.tensor_copy(identity[:], identity_f[:])
            ones = const_pool.tile([128, 4], F32)
            nc.gpsimd.memset(ones[:], 1.0)

            batches = [(0, 1), (1, 3), (4, 4), (8, 4), (12, 4)]
            for s0, bn in batches:
                TB = bn * KT
                xn = xn_pool.tile([128, TT, DW], R32, tag="xn")
                nc.gpsimd.dma_start(
                    out=xn[:, 0:TB, 0:D],
                    in_=x[s0 : s0 + bn].rearrange(
                        "s (t p) d -> p (s t) d", p=128
                    ),
                )
                # ones column for the fused softmax denominator (DVE writes
                # f32r, as the f32r matmul verifier demands)
                for tt in range(TB):
                    nc.vector.tensor_copy(xn[:, tt, D : D + 4], ones[:])

                # bf16 copy of X for the Q/K branch (f32 view for the cast),
                # chunked per position tile so transposes can start early
                xb = xb_pool.tile([128, TT, D], BF16, tag="xb")
                for tt in range(TB):
                    src_ap = xn[:, tt, 0:D].bitcast(F32)
                    if tt % 2 == 1:
                        nc.scalar.copy(xb[:, tt, :], src_ap)
                    else:
                        nc.vector.tensor_copy(xb[:, tt, :], src_ap)

                osb = osb_pool.tile([128, TT, D], F32, tag="osb")
                # ---- Q/K phase: transposes + S + exp for the whole batch
                # (pure-bf16 PE stream; ScalarE exps not queued behind evicts)
                es_all = []
                for sl in range(bn):
                    xts = []
                    for d in range(DT):
                        xt = xt_pool.tile([128, L], BF16)
                        tp = tp_pool.tile([128, L], BF16)
                        for t in range(KT):
                            nc.tensor.transpose(
                                tp[:, t * 128 : (t + 1) * 128],
                                xb[:, sl * KT + t, d * 128 : (d + 1) * 128],
                                identity[:],
                            )
                        if d >= 4:
                            nc.scalar.copy(xt[:], tp[:])
                        else:
                            nc.vector.tensor_copy(xt[:], tp[:])
                        xts.append(xt)

                    es = []
                    for kt in range(KT):
                        sp = s_pool.tile([128, L], F32)
                        for d in range(DT):
                            nc.tensor.matmul(
                                sp[:],
                                xts[d][:, kt * 128 : (kt + 1) * 128],
                                xts[d][:],
                                start=(d == 0),
                                stop=(d == DT - 1),
                            )
                        e = e_pool.tile([128, L], R32)
                        nc.scalar.activation(
                            e[:], sp[:], mybir.ActivationFunctionType.Exp, scale=SCALE
                        )
                        es.append(e)
                    es_all.append(es)

                # ---- V phase: out matmuls + normalize + store per segment
                for sl in range(bn):
                    es = es_all[sl]
                    for qt in range(KT):
                        ops = []
                        for h, (hs, hn) in enumerate([(0, 384), (384, 388)]):
                            op = out_pool.tile([128, 388], F32)
                            for kt in range(KT):
                                nc.tensor.matmul(
                                    op[:, 0:hn],
                                    es[kt][:, qt * 128 : (qt + 1) * 128],
                                    xn[:, sl * KT + kt, hs : hs + hn],
                                    start=(kt == 0),
                                    stop=(kt == KT - 1),
                                )
                            ops.append(op)
                        recip = recip_pool.tile([128, 1], F32)
                        nc.vector.reciprocal(recip[:], ops[1][:, 384:385])
                        dst0 = osb[:, sl * KT + qt, 0:384]
                        dst1 = osb[:, sl * KT + qt, 384:768]
                        nc.scalar.activation(
                            dst0,
                            ops[0][:, 0:384],
                            mybir.ActivationFunctionType.Copy,
                            scale=recip[:],
                        )
                        nc.vector.tensor_scalar_mul(dst1, ops[1][:, 0:384], recip[:])

                    nc.gpsimd.dma_start(
                        out=y[s0 + sl].rearrange("(t p) d -> p t d", p=128),
                        in_=osb[:, sl * KT : (sl + 1) * KT, :],
                    )
    return nc


def split_excess_waits(nc, max_waits=1):
    """This walrus build only encodes one sync wait per instruction; move
    excess waits onto preceding same-engine NOPs."""
    n_split = 0
    for fn in nc.m.functions:
        for blk in fn.blocks:
            insts = blk.instructions
            i = 0
            while i < len(insts):
                inst = insts[i]
                si = getattr(inst, "sync_info", None)
                waits = list(si.on_wait) if si and si.on_wait else []
                if len(waits) > max_waits:
                    nop = mybir.InstNoOp(name=f"I-waitsplit-{n_split}", ins=[], outs=[])
                    nop.engine = inst.engine
                    nop.sync_info = mybir.SyncInfo(
                        on_wait=waits[:max_waits], on_update=[]
                    )
                    inst.sync_info = mybir.SyncInfo(
                        on_wait=waits[max_waits:], on_update=list(si.on_update)
                    )
                    insts.insert(i, nop)
                    n_split += 1
                else:
                    i += 1
    return n_split


_NC = None


def _get_nc():
    global _NC
    if _NC is None:
        _NC = build_nc()
        split_excess_waits(_NC)
    return _NC


def shard_inputs(x):
    """Full x [4, 16384, 768] -> 8 per-core dicts of [16, 256, 768] (dilated)."""
    xs = np.asarray(x).reshape(B, S_FULL // SEG, SEG, D)[:, :, ::DIL, :]
    xs = xs.reshape(NSEG, L, D)
    return [
        {"x": np.ascontiguousarray(xs[SEG_PER_CORE * c : SEG_PER_CORE * (c + 1)])}
        for c in range(NCORE)
    ]


def assemble_output(results):
    out = np.concatenate([results[c]["y"] for c in range(NCORE)], axis=0)
    return out.reshape(B, NSEG // B * L, D)


def kernel(x):
    nc = _get_nc()
    in_maps = shard_inputs(x)
    core_ids = list(range(NCORE))
    # run twice: the first execution after a fresh NEFF load has been seen
    # returning unwritten output buffers; the repeat is cheap and reliable.
    run_bass_kernel_spmd(nc, in_maps, core_ids)
    res = run_bass_kernel_spmd(nc, in_maps, core_ids)
    return assemble_output(res.results).astype(np.float32)

